# revision 7
# baseline (speedup 1.0000x reference)
"""Distributed causal multi-head attention (Bass/Tile, 8 TRN2 NeuronCores).

Sharding: core = (batch b, rank r) with b = core // 4, r = core % 4.
Within a batch group of 4 cores, rank r owns query rows {g : g % 4 == r}
(row-interleaved sequence parallelism).  Every core runs the IDENTICAL
graph; rank-dependence lives entirely in the input data (x^T shard and a
small diagonal-mask tensor built on the host).

Per core:
  q^T, k^T = (x_own @ Wq/Wk)^T   [C, 512]   (transposed orientation)
  v        =  x_own @ Wv         [512, C]   (normal orientation)
  AllGather (bf16) of packed [k^T | v] across the 4 ranks
  scores[tq, tk] = q^T.T @ k^T chunks  (keys in rank-permuted order)
  softmax: fused exp + row-sum via activation(accum_out), per-partition
  normalize, PE-transpose att tiles, AV matmul (2 heads col-packed)
  producing out^T directly, then y = out^T.T @ Wo.
"""

import numpy as np

B, T, C, H = 2, 2048, 1024, 16
D = C // H            # 64
R = 4                 # ranks per batch group
TOWN = T // R         # 512 rows owned per core
NJ = T // 512         # 4 key 512-chunks
NT = TOWN // 128      # 4 local query 128-tiles
CC = C // 128         # 8 contraction chunks
PAIRS = H // 2        # 8 head pairs
KT_ELEMS = C * TOWN   # k^T shard elems
V_ELEMS = TOWN * C    # v shard elems
AG_ELEMS = KT_ELEMS + V_ELEMS
SCALE = 1.0 / 32.0    # 1/sqrt(C)
NEG = -1e30

_cached_nc = None
last_result = None
_DEBUG = False


def _dbg(nc, P, col, ap, width):
    if P.get("dbg_ext") is not None:
        nc.sync.dma_start(P["dbg_ext"][:, col : col + width], ap)


def _qkv_phase(nc, P, mybir):
    """k^T, v, bounce DMAs + AllGather trigger, then q^T."""
    F32, BF16 = mybir.dt.float32, mybir.dt.bfloat16
    wqkv_sb, xt_sb = P["wqkv_sb"], P["xt_sb"]
    mm_p = P["mm_p"]

    groups = [[0, 1, 2, 3], [4, 5, 6, 7]]
    kt_loc = P["kv_p"].tile([128, CC * TOWN], BF16, tag="kt_loc")
    for kc in range(CC):
        ps = mm_p.tile([128, 512], F32, tag="mm")
        for cc in range(CC):
            nc.tensor.matmul(
                ps[:],
                wqkv_sb[:, cc * 3 * C + C + kc * 128 : cc * 3 * C + C + (kc + 1) * 128],
                xt_sb[:, cc * TOWN : (cc + 1) * TOWN],
                start=(cc == 0),
                stop=(cc == CC - 1),
            )
        nc.vector.tensor_copy(kt_loc[:, kc * TOWN : (kc + 1) * TOWN], ps[:])
    _dbg(nc, P, 512, kt_loc[:, 0:512], 512)

    bounce_kt = P["dram_p"].tile([KT_ELEMS], BF16, tag="bounce_kt")
    for kc in range(CC):
        nc.sync.dma_start(
            bounce_kt[kc * TOWN * 128 : (kc + 1) * TOWN * 128].rearrange("(p f) -> p f", p=128),
            kt_loc[:, kc * TOWN : (kc + 1) * TOWN],
        )
    gathered_kt = P["dram_p"].tile([R * KT_ELEMS], BF16, tag="gathered_kt")
    nc.gpsimd.collective_compute(
        "AllGather",
        mybir.AluOpType.bypass,
        replica_groups=groups,
        ins=[bounce_kt.opt()],
        outs=[gathered_kt.opt()],
    )
    P["gathered_kt"] = gathered_kt

    v_loc = P["kv_p"].tile([128, NT * C], BF16, tag="v_loc")
    for t in range(NT):
        for hf in range(2):
            ps = mm_p.tile([128, 512], F32, tag="mm")
            for cc in range(CC):
                nc.tensor.matmul(
                    ps[:],
                    xt_sb[:, cc * TOWN + t * 128 : cc * TOWN + (t + 1) * 128],
                    wqkv_sb[:, cc * 3 * C + 2 * C + hf * 512 : cc * 3 * C + 2 * C + (hf + 1) * 512],
                    start=(cc == 0),
                    stop=(cc == CC - 1),
                )
            nc.vector.tensor_copy(v_loc[:, t * C + hf * 512 : t * C + (hf + 1) * 512], ps[:])

    bounce_v = P["dram_p"].tile([V_ELEMS], BF16, tag="bounce_v")
    for t in range(NT):
        nc.sync.dma_start(
            bounce_v[t * C * 128 : (t + 1) * C * 128].rearrange("(p f) -> p f", p=128),
            v_loc[:, t * C : (t + 1) * C],
        )
    gathered_v = P["dram_p"].tile([R * V_ELEMS], BF16, tag="gathered_v")
    nc.gpsimd.collective_compute(
        "AllGather",
        mybir.AluOpType.bypass,
        replica_groups=groups,
        ins=[bounce_v.opt()],
        outs=[gathered_v.opt()],
    )
    P["gathered_v"] = gathered_v

    qt_sb = P["qt_p"].tile([128, CC * TOWN], BF16, tag="qt")
    for qc in range(CC):
        ps = mm_p.tile([128, 512], F32, tag="mm")
        for cc in range(CC):
            nc.tensor.matmul(
                ps[:],
                wqkv_sb[:, cc * 3 * C + qc * 128 : cc * 3 * C + (qc + 1) * 128],
                xt_sb[:, cc * TOWN : (cc + 1) * TOWN],
                start=(cc == 0),
                stop=(cc == CC - 1),
            )
        nc.vector.tensor_scalar_mul(qt_sb[:, qc * TOWN : (qc + 1) * TOWN], ps[:], SCALE)
    P["qt_sb"] = qt_sb
    _dbg(nc, P, 0, qt_sb[:, 0:512], 512)


def _gather_kv_pair(nc, P, p, mybir):
    """Load this head pair's gathered k^T and v into SBUF."""
    BF16 = mybir.dt.bfloat16
    gathered_kt, gathered_v = P["gathered_kt"], P["gathered_v"]
    ktg = P["ktg_p"].tile([128, 2048], BF16, tag="ktg")
    for s in range(R):
        src = gathered_kt[
            s * KT_ELEMS + p * 128 * TOWN : s * KT_ELEMS + (p + 1) * 128 * TOWN
        ].rearrange("(q f) -> q f", q=128)
        nc.sync.dma_start(
            ktg[:].rearrange("q (j g) -> q j g", g=512)[:, :, s * 128 : (s + 1) * 128],
            src.rearrange("q (j i) -> q j i", i=128),
        )
    vg = P["vg_p"].tile([128, 2048], BF16, tag="vg")
    for s in range(R):
        src = gathered_v[
            s * V_ELEMS : (s + 1) * V_ELEMS
        ].rearrange("(m c) -> m c", c=C)[:, p * 128 : (p + 1) * 128]
        nc.sync.dma_start(
            vg[:].rearrange("i (j g) -> i j g", g=512)[:, :, s * 128 : (s + 1) * 128],
            src.rearrange("(j i) c -> i j c", i=128),
        )
    if p == 0:
        _dbg(nc, P, 1024, ktg[:], 2048)
        _dbg(nc, P, 3072, vg[:], 2048)
    return ktg, vg


def _scores_softmax(nc, P, p, ktg, mybir):
    """Scores + fused exp/accum softmax for both heads of pair p."""
    F32, BF16 = mybir.dt.float32, mybir.dt.bfloat16
    AFT = mybir.ActivationFunctionType
    qt_sb, dmask = P["qt_sb"], P["dmask"]
    mm_p, att_p, sm_p = P["mm_p"], P["att_p"], P["sm_p"]

    atts = [[None] * NT, [None] * NT]
    for t in range(NT):
        att0 = att_p.tile([128, 2048], BF16, tag="att")
        att1 = att_p.tile([128, 2048], BF16, tag="att")
        atts[0][t], atts[1][t] = att0, att1
        parts0 = sm_p.tile([128, NT], F32, tag="parts")
        parts1 = sm_p.tile([128, NT], F32, tag="parts")
        prefix = (t + 1) * 512
        nch = (prefix + 1023) // 1024
        for ch in range(nch):
            width = min(1024, prefix - ch * 1024)
            ps0 = mm_p.tile([128, 1024], F32, tag="mm")
            ps1 = mm_p.tile([128, 1024], F32, tag="mm")
            for sub in range(width // 512):
                jj = ch * 2 + sub
                nc.tensor.matmul(
                    ps0[:, sub * 512 : (sub + 1) * 512],
                    qt_sb[0:64, p * TOWN + t * 128 : p * TOWN + (t + 1) * 128],
                    ktg[0:64, jj * 512 : (jj + 1) * 512],
                    start=True,
                    stop=True,
                )
                nc.tensor.matmul(
                    ps1[:, sub * 512 : (sub + 1) * 512],
                    qt_sb[64:128, p * TOWN + t * 128 : p * TOWN + (t + 1) * 128],
                    ktg[64:128, jj * 512 : (jj + 1) * 512],
                    start=True,
                    stop=True,
                )
                if jj == t:
                    nc.vector.tensor_add(
                        ps0[:, sub * 512 : (sub + 1) * 512],
                        ps0[:, sub * 512 : (sub + 1) * 512], dmask[:],
                    )
                    nc.vector.tensor_add(
                        ps1[:, sub * 512 : (sub + 1) * 512],
                        ps1[:, sub * 512 : (sub + 1) * 512], dmask[:],
                    )
            nc.scalar.activation(
                att0[:, ch * 1024 : ch * 1024 + width], ps0[:, :width], AFT.Exp,
                accum_out=parts0[:, ch : ch + 1],
            )
            nc.scalar.activation(
                att1[:, ch * 1024 : ch * 1024 + width], ps1[:, :width], AFT.Exp,
                accum_out=parts1[:, ch : ch + 1],
            )
        for hh, parts in ((0, parts0), (1, parts1)):
            den = sm_p.tile([128, 1], F32, tag="den")
            nc.vector.reduce_sum(den[:], parts[:, : nch], axis=mybir.AxisListType.X)
            rec = sm_p.tile([128, 1], F32, tag="rec")
            nc.vector.reciprocal(rec[:], den[:])
            att = atts[hh][t]
            nc.vector.tensor_scalar_mul(att[:, : (t + 1) * 512], att[:, : (t + 1) * 512], rec[:])
    if p == 0:
        _dbg(nc, P, 5120, atts[0][0][:, 0:512], 512)
        _dbg(nc, P, 5632, atts[0][3][:, 0:2048], 2048)
    return atts


def _av_phase(nc, P, p, atts, vg, mybir):
    """Transpose att tiles on PE and run the AV matmul (out^T, col-packed)."""
    F32, BF16 = mybir.dt.float32, mybir.dt.bfloat16
    ident = P["ident"]
    avp0 = P["av_p"].tile([128, TOWN], F32, tag="av")
    avp1 = P["av_p"].tile([128, TOWN], F32, tag="av")
    avps = [avp0, avp1]
    for hh in range(2):
        avp = avps[hh]
        first = True
        for jj in range(NJ):
            for s in range(R):
                trp = P["tr_p"].tile([128, 512], BF16, tag="tr")
                for t in range(jj, NT):
                    nc.tensor.transpose(
                        trp[:, t * 128 : (t + 1) * 128],
                        atts[hh][t][:, jj * 512 + s * 128 : jj * 512 + (s + 1) * 128],
                        ident[:],
                    )
                attT = P["attT_p"].tile([128, 512], BF16, tag="attT")
                nc.vector.tensor_copy(attT[:, jj * 128 :], trp[:, jj * 128 :])
                if p == 0 and hh == 0 and jj == 0:
                    _dbg(nc, P, 8192 + s * 512, attT[:, :], 512)
                nc.tensor.matmul(
                    avp[hh * 64 : (hh + 1) * 64, jj * 128 :],
                    vg[:, (jj * R + s) * 128 + hh * 64 : (jj * R + s) * 128 + (hh + 1) * 64],
                    attT[:, jj * 128 :],
                    start=first,
                    stop=(jj == NJ - 1 and s == R - 1),
                    tile_position=(0, hh * 64),
                )
                first = False
        nc.vector.tensor_copy(
            P["outT_sb"][hh * 64 : (hh + 1) * 64, p * TOWN : (p + 1) * TOWN],
            avp[hh * 64 : (hh + 1) * 64, :],
        )
    if p == 0:
        _dbg(nc, P, 7680, P["outT_sb"][:, 0:512], 512)


def _wo_phase(nc, P, mybir):
    F32 = mybir.dt.float32
    outT_sb, wo_sb, mm_p = P["outT_sb"], P["wo_sb"], P["mm_p"]
    y_sb = P["y_p"].tile([128, NT * C], F32, tag="y")
    for t in range(NT):
        for hf in range(2):
            ps = mm_p.tile([128, 512], F32, tag="mm")
            for cc in range(CC):
                nc.tensor.matmul(
                    ps[:],
                    outT_sb[:, cc * TOWN + t * 128 : cc * TOWN + (t + 1) * 128],
                    wo_sb[:, cc * C + hf * 512 : cc * C + (hf + 1) * 512],
                    start=(cc == 0),
                    stop=(cc == CC - 1),
                )
            nc.vector.tensor_copy(y_sb[:, t * C + hf * 512 : t * C + (hf + 1) * 512], ps[:])
    for t in range(NT):
        nc.sync.dma_start(P["out_ext"][t * 128 : (t + 1) * 128, :], y_sb[:, t * C : (t + 1) * C])


def _body(nc, P, mybir):
    from concourse.masks import make_identity
    from concourse.bass import ts

    F32, BF16 = mybir.dt.float32, mybir.dt.bfloat16

    ident = P["const_p"].tile([128, 128], BF16, tag="ident")
    make_identity(nc, ident[:])
    P["ident"] = ident
    dmask = P["const_p"].tile([128, 512], F32, tag="dmask")
    nc.sync.dma_start(dmask[:], P["dmask_ext"][:])
    P["dmask"] = dmask

    wqkv_sb = P["w_p"].tile([128, CC * 3 * C], BF16, tag="wqkv")
    for cc in range(CC):
        nc.sync.dma_start(wqkv_sb[:, cc * 3 * C : (cc + 1) * 3 * C], P["wqkv_ext"][ts(cc, 128), :])
    P["wqkv_sb"] = wqkv_sb
    wo_sb = P["w_p"].tile([128, CC * C], BF16, tag="wo")
    for cc in range(CC):
        nc.sync.dma_start(wo_sb[:, cc * C : (cc + 1) * C], P["wo_ext"][ts(cc, 128), :])
    P["wo_sb"] = wo_sb
    xt_sb = P["x_p"].tile([128, CC * TOWN], BF16, tag="xt")
    for cc in range(CC):
        nc.sync.dma_start(xt_sb[:, cc * TOWN : (cc + 1) * TOWN], P["xt_ext"][ts(cc, 128), :])
    P["xt_sb"] = xt_sb

    _qkv_phase(nc, P, mybir)

    outT_sb = P["outT_p"].tile([128, PAIRS * TOWN], BF16, tag="outT")
    P["outT_sb"] = outT_sb
    for p in range(PAIRS):
        ktg, vg = _gather_kv_pair(nc, P, p, mybir)
        atts = _scores_softmax(nc, P, p, ktg, mybir)
        _av_phase(nc, P, p, atts, vg, mybir)

    _wo_phase(nc, P, mybir)


def _build():
    import concourse.mybir as mybir
    import concourse.tile as tile
    from concourse import bacc

    F32, BF16 = mybir.dt.float32, mybir.dt.bfloat16

    nc = bacc.Bacc("TRN2", target_bir_lowering=False, debug=False, num_devices=8)
    P = {
        "xt_ext": nc.declare_dram_parameter("xt", [C, TOWN], BF16, isOutput=False),
        "wqkv_ext": nc.declare_dram_parameter("wqkv", [C, 3 * C], BF16, isOutput=False),
        "wo_ext": nc.declare_dram_parameter("wo", [C, C], BF16, isOutput=False),
        "dmask_ext": nc.declare_dram_parameter("dmask", [128, 512], F32, isOutput=False),
        "out_ext": nc.declare_dram_parameter("out", [TOWN, C], F32, isOutput=True),
    }
    if _DEBUG:
        P["dbg_ext"] = nc.declare_dram_parameter("dbg", [128, 10240], BF16, isOutput=True)

    with tile.TileContext(nc) as tc:
        with (
            tc.tile_pool(name="const", bufs=1) as const_p,
            tc.tile_pool(name="w", bufs=1) as w_p,
            tc.tile_pool(name="x", bufs=1) as x_p,
            tc.tile_pool(name="qt", bufs=1) as qt_p,
            tc.tile_pool(name="kv", bufs=1) as kv_p,
            tc.tile_pool(name="att", bufs=10) as att_p,
            tc.tile_pool(name="ktg", bufs=2) as ktg_p,
            tc.tile_pool(name="vg", bufs=2) as vg_p,
            tc.tile_pool(name="attT", bufs=4) as attT_p,
            tc.tile_pool(name="outT", bufs=1) as outT_p,
            tc.tile_pool(name="y", bufs=1) as y_p,
            tc.tile_pool(name="sm", bufs=16) as sm_p,
            tc.tile_pool(name="mmp", bufs=2, space="PSUM") as mm_p,
            tc.tile_pool(name="avp", bufs=2, space="PSUM") as av_p,
            tc.tile_pool(name="trp", bufs=2, space="PSUM") as tr_p,
            tc.tile_pool(name="dram", bufs=1, space="DRAM") as dram_p,
        ):
            P.update(
                const_p=const_p, w_p=w_p, x_p=x_p, qt_p=qt_p, kv_p=kv_p,
                att_p=att_p, ktg_p=ktg_p, vg_p=vg_p, attT_p=attT_p,
                outT_p=outT_p, y_p=y_p, sm_p=sm_p, mm_p=mm_p, av_p=av_p,
                tr_p=tr_p, dram_p=dram_p,
            )
            _body(nc, P, mybir)

    nc.finalize()
    return nc


def kernel(x, Wqkv, bqkv, Wo, bo):
    global _cached_nc, last_result
    import ml_dtypes
    from concourse.bass_utils import run_bass_kernel_spmd

    if _cached_nc is None:
        _cached_nc = _build()
    nc = _cached_nc

    bf16 = ml_dtypes.bfloat16
    x = np.asarray(x, dtype=np.float32)
    wq_b = np.ascontiguousarray(np.asarray(Wqkv, dtype=np.float32).astype(bf16))
    wo_b = np.ascontiguousarray(np.asarray(Wo, dtype=np.float32).astype(bf16))

    # diagonal-chunk causal mask, per rank r: key (i, s) vs query partition p
    i_idx = np.arange(128)[None, None, :]
    p_idx = np.arange(128)[:, None, None]
    s_idx = np.arange(R)[None, :, None]

    in_maps = []
    for core in range(8):
        b, r = divmod(core, R)
        xt = np.ascontiguousarray(x[b].T[:, r::R].astype(bf16))
        masked = (i_idx > p_idx) | ((i_idx == p_idx) & (s_idx > r))
        dm = np.where(masked, np.float32(NEG), np.float32(0.0)).reshape(128, 512)
        in_maps.append(
            {"xt": xt, "wqkv": wq_b, "wo": wo_b, "dmask": np.ascontiguousarray(dm)}
        )

    last_result = run_bass_kernel_spmd(nc, in_maps, core_ids=list(range(8)))

    y = np.empty((B, T, C), dtype=np.float32)
    for core in range(8):
        b, r = divmod(core, R)
        y[b, r::R, :] = last_result.results[core]["out"]
    return y
